# revision 1
# baseline (speedup 1.0000x reference)
"""Trainium2 Bass kernel for nn_ActorNetwork (gnn_message_passing).

Pure data-parallel across 8 NeuronCores: each core processes 8192 of the
65536 batch rows; small weights are replicated.

Math (per row b):
  own  = relu(state0 @ W0 + b0)                       [64]
  env  = relu(state1 @ Wg + bg)                       [64]
  i1   = relu(state2 @ Ws1 + bs1)                     [8, 64]
  i2   = relu(i1 @ Ws2 + bs2)                         [8, 64]
  score_n = i2_n . (own @ (Wq Wk^T))                  (k-matmul folded away)
  alpha = masked softmax(score / 8)
  c    = sum_n alpha_n i2_n                           [64]
  v_att@Wc1[128:] = c @ (Wv Wc1[128:])                (v-matmul folded away)
  h1 = relu([own, env] @ Wc1[:128] + c @ Wvc + bc1)
  h2 = relu(h1 @ Wc2 + bc2);  out = tanh(h2 @ Wc3 + bc3)

Layout: feature-major ("X^T" = [features, batch]) so every matmul keeps the
small weights stationary and streams wide (N=512) moving operands at
1 cyc/col (bf16 or float32r).  state1 is cast f32->bf16 during the SWDGE DMA
load and transposed on-chip with the DMA xbar (free of compute engines).
The attention softmax runs batch-major ([128 rows, 8 slots]) after cheap PE
transposes of i2 / q'.
"""

import os

import numpy as np
import ml_dtypes

import concourse.bass as bass
import concourse.tile as tile
from concourse import bacc
from concourse import mybir
from concourse.bass_utils import run_bass_kernel_spmd
from concourse.masks import make_identity

F32 = mybir.dt.float32
F32R = mybir.dt.float32r
BF16 = mybir.dt.bfloat16

N_CORES = 8
B_FULL = 65536
RPC = B_FULL // N_CORES        # rows per core = 8192
MACRO = 512                    # batch rows per macro tile
P = 128

Relu = mybir.ActivationFunctionType.Relu
Tanh = mybir.ActivationFunctionType.Tanh
Exp = mybir.ActivationFunctionType.Exp
Copy = mybir.ActivationFunctionType.Copy
Alu = mybir.AluOpType
AX = mybir.AxisListType


def _r(ap):
    return ap.bitcast(F32R)


def build(rpc=RPC, macro=MACRO):
    STAGE = int(os.environ.get("K_STAGE", "9"))
    nm = rpc // macro          # macro tiles per core
    nb = macro // P            # 128-row blocks per macro tile

    nc = bacc.Bacc()

    s0 = nc.declare_dram_parameter("state0", [rpc, 6], F32R, isOutput=False)
    s1 = nc.declare_dram_parameter("state1", [rpc, 1024], F32, isOutput=False)
    s2 = nc.declare_dram_parameter("state2", [rpc, 8, 7], F32, isOutput=False)
    wg_d = nc.declare_dram_parameter("wg_bf", [1024, 64], BF16, isOutput=False)
    ws1_d = nc.declare_dram_parameter("ws1_blk", [28, 2, 128], BF16, isOutput=False)
    ws2_d = nc.declare_dram_parameter("ws2_blk", [128, 128], BF16, isOutput=False)
    w0_d = nc.declare_dram_parameter("w0", [6, 64], F32R, isOutput=False)
    wqk_d = nc.declare_dram_parameter("wqk", [64, 64], F32R, isOutput=False)
    wc1a_d = nc.declare_dram_parameter("wc1a", [128, 128], F32R, isOutput=False)
    wvc_d = nc.declare_dram_parameter("wvc", [64, 128], F32R, isOutput=False)
    wc2_d = nc.declare_dram_parameter("wc2", [128, 128], F32R, isOutput=False)
    wc3_d = nc.declare_dram_parameter("wc3", [128, 2], F32R, isOutput=False)
    b0bg_d = nc.declare_dram_parameter("b0bg", [128, 1], F32, isOutput=False)
    bs1_d = nc.declare_dram_parameter("bs1_rep", [128, 1], F32, isOutput=False)
    bs2_d = nc.declare_dram_parameter("bs2_rep", [128, 1], F32, isOutput=False)
    bc1_d = nc.declare_dram_parameter("bc1", [128, 1], F32, isOutput=False)
    bc2_d = nc.declare_dram_parameter("bc2", [128, 1], F32, isOutput=False)
    bc3_d = nc.declare_dram_parameter("bc3", [2, 1], F32, isOutput=False)
    out_d = nc.declare_dram_parameter("out", [2, rpc], F32, isOutput=True)

    with tile.TileContext(nc) as tc:
        consts = tc.alloc_tile_pool(name="consts", bufs=1)
        s1stage_p = tc.alloc_tile_pool(name="s1stage", bufs=2)
        s1T_p = tc.alloc_tile_pool(name="s1T", bufs=2)
        s2_p = tc.alloc_tile_pool(name="s2", bufs=2)
        s2c_p = tc.alloc_tile_pool(name="s2c", bufs=3)
        s2T_p = tc.alloc_tile_pool(name="s2T", bufs=3)
        work_p = tc.alloc_tile_pool(name="work", bufs=2)
        blk_p = tc.alloc_tile_pool(name="blk", bufs=3)
        sm_p = tc.alloc_tile_pool(name="sm", bufs=3)
        psA = tc.alloc_tile_pool(name="psA", bufs=2, space="PSUM")
        psB = tc.alloc_tile_pool(name="psB", bufs=2, space="PSUM")
        psS = tc.alloc_tile_pool(name="psS", bufs=2, space="PSUM")
        psC = tc.alloc_tile_pool(name="psC", bufs=2, space="PSUM")

        # ---- constants / weights to SBUF ----
        wg_sb = consts.tile([P, 8, 64], BF16)
        nc.sync.dma_start(wg_sb, wg_d.rearrange("(c p) m -> p c m", p=P))
        ws1_sb = consts.tile([28, 2, P], BF16)
        nc.sync.dma_start(ws1_sb, ws1_d[:, :, :])
        ws2_sb = consts.tile([P, P], BF16)
        nc.sync.dma_start(ws2_sb, ws2_d[:, :])
        w0_sb = consts.tile([6, 64], F32R)
        nc.sync.dma_start(w0_sb, w0_d[:, :])
        wqk_sb = consts.tile([64, 64], F32R)
        nc.sync.dma_start(wqk_sb, wqk_d[:, :])
        wc1a_sb = consts.tile([P, 128], F32R)
        nc.sync.dma_start(wc1a_sb, wc1a_d[:, :])
        wvc_sb = consts.tile([64, 128], F32R)
        nc.sync.dma_start(wvc_sb, wvc_d[:, :])
        wc2_sb = consts.tile([P, 128], F32R)
        nc.sync.dma_start(wc2_sb, wc2_d[:, :])
        wc3_sb = consts.tile([P, 2], F32R)
        nc.sync.dma_start(wc3_sb, wc3_d[:, :])

        b0bg_sb = consts.tile([P, 1], F32)
        nc.sync.dma_start(b0bg_sb, b0bg_d[:, :])
        bs1_sb = consts.tile([P, 1], F32)
        nc.sync.dma_start(bs1_sb, bs1_d[:, :])
        bs2_sb = consts.tile([P, 1], F32)
        nc.sync.dma_start(bs2_sb, bs2_d[:, :])
        bc1_sb = consts.tile([P, 1], F32)
        nc.sync.dma_start(bc1_sb, bc1_d[:, :])
        bc2_sb = consts.tile([P, 1], F32)
        nc.sync.dma_start(bc2_sb, bc2_d[:, :])
        bc3_sb = consts.tile([2, 1], F32)
        nc.sync.dma_start(bc3_sb, bc3_d[:, :])

        # state0^T loaded once for the whole core (tiny, strided DMA)
        s0T_sb = consts.tile([6, rpc], F32R)
        if STAGE >= 1:
            with nc.allow_non_contiguous_dma(reason="tiny state0 transpose load"):
                nc.sync.dma_start(s0T_sb, s0.rearrange("b f -> f b"))

        ident_f = consts.tile([P, P], F32)
        make_identity(nc, ident_f)
        ident = consts.tile([P, P], F32R)
        nc.vector.tensor_copy(ident, ident_f)
        ident_b = consts.tile([P, P], BF16)
        nc.vector.tensor_copy(ident_b, ident_f)

        out_sb = consts.tile([2, rpc], F32)
        if STAGE < 9:
            nc.gpsimd.memset(out_sb, 0.0)

        for ms in range(nm):
            row0 = ms * macro
            # ---- streamed loads ----
            s1_stage = s1stage_p.tile([P, nb, 1024], BF16, tag="s1stage")
            nc.gpsimd.dma_start(
                s1_stage,
                s1[row0 : row0 + macro, :].rearrange("(o p) f -> p o f", p=P),
            )
            s2_t = s2_p.tile([P, nb, 8, 7], F32, tag="s2")
            nc.sync.dma_start(
                s2_t,
                s2[row0 : row0 + macro, :, :].rearrange("(o p) n j -> p o n j", p=P),
            )
            # state1 transpose on PE (bf16): s1T[:, c, o*128+b] = s1[b, c*128+f]
            s1T = s1T_p.tile([P, 8, macro], BF16, tag="s1T")
            for o in range(nb if STAGE >= 2 else 0):
                for half in range(2):
                    t_ps = psS.tile([P, 4, P], BF16, tag="psS")
                    for cc in range(4):
                        c = 4 * half + cc
                        nc.tensor.transpose(
                            t_ps[:, cc, :],
                            s1_stage[:, o, c * P : (c + 1) * P],
                            ident_b,
                        )
                    dst = s1T[:, 4 * half : 4 * half + 4, o * P : (o + 1) * P]
                    if half == 0:
                        nc.vector.tensor_copy(dst, t_ps)
                    else:
                        nc.scalar.copy(dst, t_ps)

            # ---- own_e / env_e (packed into one [128, 512] psum) ----
            if STAGE < 3:
                continue
            eo_ps = psB.tile([P, macro], F32, tag="psB")
            nc.tensor.matmul(
                eo_ps[0:64, :], w0_sb, s0T_sb[:, row0 : row0 + macro],
                start=True, stop=True,
            )
            for c in range(8):
                nc.tensor.matmul(
                    eo_ps[64:128, :], wg_sb[:, c], s1T[:, c, :],
                    start=(c == 0), stop=(c == 7), tile_position=(0, 64),
                )
            concatA = work_p.tile([P, macro], F32R, tag="concatA")
            nc.scalar.activation(concatA, eo_ps, Relu, bias=b0bg_sb)

            # ---- q' = own @ (Wq Wk^T) ----
            q_ps = psB.tile([P, macro], F32, tag="psB")
            nc.tensor.matmul(
                q_ps[0:64, :], wqk_sb, concatA[0:64, :],
                start=True, stop=True,
            )
            q_sb = work_p.tile([64, macro], F32R, tag="q_sb")
            nc.vector.tensor_copy(q_sb, q_ps[0:64, :])

            # ---- mask from exact f32 state2 ----
            msum = sm_p.tile([P, nb, 8], F32, tag="msum")
            nc.vector.reduce_sum(msum, s2_t, axis=AX.X)
            negm = sm_p.tile([P, nb, 8], F32, tag="negm")
            nc.vector.tensor_scalar(negm, msum, 0.0, -1e30, Alu.is_equal, Alu.mult)

            cT_sb = work_p.tile([64, macro], F32R, tag="cT")
            score_m = sm_p.tile([P, nb, 8], F32, tag="score")
            kv_list = []

            if STAGE < 4:
                continue
            SUB = int(os.environ.get("K_SUB", "9"))
            for o in range(nb):
                # ---- state2 -> bf16, xbar transpose to [j-in-32-groups, b] ----
                s2c = s2c_p.tile([P, 2, P], BF16, tag="s2c")
                nc.gpsimd.memset(s2c, 0.0)
                nc.vector.tensor_copy(
                    s2c[:, :, 0:28].rearrange("p t (a b) -> p t a b", b=7),
                    s2_t[:, o].rearrange("p (t a) b -> p t a b", t=2),
                )
                s2T = s2T_p.tile([28, 2, P], BF16, tag="s2T")
                s2t_ps = psS.tile([P, 4, P], BF16, tag="psS")
                for t in range(2):
                    nc.tensor.transpose(
                        s2t_ps[:, t, :],
                        s2c[:, t, :],
                        ident_b,
                    )
                nc.scalar.copy(s2T, s2t_ps[0:28, 0:2])

                if SUB < 2:
                    continue
                # ---- intru1: 4 plain matmuls, block-diag Ws1 ----
                i1_ps = psA.tile([P, macro], F32, tag="psA")
                for t in range(2):
                    for q in range(2):
                        m = 2 * t + q
                        nc.tensor.matmul(
                            i1_ps[:, 128 * m : 128 * m + 128],
                            ws1_sb[:, q, :],
                            s2T[:, t, :],
                            start=True, stop=True,
                        )
                i1_sb = blk_p.tile([P, macro], BF16, tag="i1")
                if o % 2 == 0:
                    nc.vector.tensor_scalar(
                        i1_sb, i1_ps, bs1_sb, 0.0, Alu.add, Alu.max
                    )
                else:
                    nc.scalar.activation(i1_sb, i1_ps, Relu, bias=bs1_sb)

                # ---- intru2: 2 packed matmuls ----
                if SUB < 3:
                    continue
                i2_ps = psA.tile([P, macro], F32, tag="psA")
                for m in range(4):
                    nc.tensor.matmul(
                        i2_ps[:, 128 * m : 128 * m + 128],
                        ws2_sb,
                        i1_sb[:, 128 * m : 128 * m + 128],
                        start=True, stop=True,
                    )
                i2_sb = blk_p.tile([P, macro], BF16, tag="i2")
                if o % 2 == 1:
                    nc.vector.tensor_scalar(
                        i2_sb, i2_ps, bs2_sb, 0.0, Alu.add, Alu.max
                    )
                else:
                    nc.scalar.activation(i2_sb, i2_ps, Relu, bias=bs2_sb)

                # ---- transpose i2 (per n) and q' to batch-major ----
                if STAGE < 5:
                    continue
                SUB5 = int(os.environ.get("K_SUB5", "9"))
                kv_ps = psB.tile([P, macro], BF16, tag="psB")
                for m in range(4):
                    nc.tensor.transpose(
                        kv_ps[:, 128 * m : 128 * m + 128],
                        i2_sb[:, 128 * m : 128 * m + 128],
                        ident_b,
                    )
                kv_sb = blk_p.tile([P, macro], BF16, tag="kv_sb", bufs=6)
                kv_list.append(kv_sb)
                nc.scalar.copy(kv_sb, kv_ps)
                if SUB5 < 2:
                    continue
                qt_ps = psC.tile([P, P], F32, tag="psC")
                nc.tensor.transpose(
                    _r(qt_ps[:, 0:64]),
                    q_sb[:, o * P : (o + 1) * P],
                    ident[0:64, 0:64],
                )
                qbm = blk_p.tile([P, 64], BF16, tag="qbm")
                nc.scalar.copy(qbm, qt_ps[:, 0:64])

                # ---- score_n = i2_n . q'  (batch-major) ----
                if SUB5 < 3:
                    continue
                qk = blk_p.tile([P, 8, 64], BF16, tag="qk")
                nc.vector.tensor_tensor(
                    qk,
                    kv_sb.rearrange("p (n d) -> p n d", n=8),
                    qbm[:, None, :].to_broadcast((P, 8, 64)),
                    Alu.mult,
                )
                nc.vector.reduce_sum(score_m[:, o], qk, axis=AX.X)

            # ---- masked softmax over the 8 slots (whole macro at once) ----
            if STAGE >= 5 and int(os.environ.get("K_SUB5", "9")) >= 4:
                # scores are O(10) for this model family, so exp(score/8)
                # cannot overflow — skip the max-subtraction pass entirely.
                sm_m = sm_p.tile([P, nb, 8], F32, tag="smt")
                nc.vector.tensor_tensor(sm_m, score_m, negm, Alu.add)
                p8_m = sm_p.tile([P, nb, 8], F32, tag="p8")
                nc.scalar.activation(p8_m, sm_m, Exp, scale=0.125)
                den_m = sm_p.tile([P, nb], F32, tag="den")
                nc.vector.reduce_sum(den_m, p8_m, axis=AX.X)
                rs_m = sm_p.tile([P, nb], F32, tag="rs")
                nc.vector.reciprocal(rs_m, den_m)
                alpha_m = sm_p.tile([P, nb, 8], F32, tag="alpha")
                nc.vector.tensor_tensor(
                    alpha_m, p8_m,
                    rs_m[:, :, None].to_broadcast((P, nb, 8)),
                    Alu.mult,
                )

                # ---- c = sum_n alpha_n * i2_n ; back to feature-major ----
                for o in range(nb):
                    cp = blk_p.tile([P, 64, 8], BF16, tag="cp")
                    nc.vector.tensor_tensor(
                        cp,
                        kv_list[o].rearrange("p (n d) -> p d n", n=8),
                        alpha_m[:, o, None, :].to_broadcast((P, 64, 8)),
                        Alu.mult,
                    )
                    c_sb = blk_p.tile([P, 64], F32R, tag="c_sb")
                    with nc.allow_low_precision(reason="f32r 19-bit; 8-term sum"):
                        nc.vector.reduce_sum(c_sb, cp, axis=AX.X)
                    ct_ps = psC.tile([P, P], F32, tag="psC")
                    nc.tensor.transpose(_r(ct_ps[0:64, :]), c_sb, ident)
                    nc.scalar.copy(cT_sb[:, o * P : (o + 1) * P], ct_ps[0:64, :])

            # ---- head: h1 = relu(Wc1a^T concatA + Wvc^T cT + bc1) ----
            if STAGE < 6:
                continue
            h1_ps = psB.tile([P, macro], F32, tag="psB")
            nc.tensor.matmul(
                h1_ps, wc1a_sb, concatA, start=True, stop=False
            )
            nc.tensor.matmul(
                h1_ps, wvc_sb, cT_sb, start=False, stop=True
            )
            h1_sb = work_p.tile([P, macro], F32R, tag="h1")
            nc.vector.tensor_scalar(
                h1_sb, h1_ps, bc1_sb, 0.0, Alu.add, Alu.max
            )

            h2_ps = psB.tile([P, macro], F32, tag="psB")
            nc.tensor.matmul(
                h2_ps, wc2_sb, h1_sb, start=True, stop=True
            )
            h2_sb = work_p.tile([P, macro], F32R, tag="h2")
            nc.scalar.activation(h2_sb, h2_ps, Relu, bias=bc2_sb)

            o_ps = psC.tile([2, macro], F32, tag="psC")
            nc.tensor.matmul(
                o_ps[0:2, :], wc3_sb, h2_sb, start=True, stop=True
            )
            nc.scalar.activation(
                out_sb[:, row0 : row0 + macro], o_ps[0:2, :], Tanh, bias=bc3_sb
            )

        nc.sync.dma_start(out_d[:, :], out_sb)

        for _pool in (psC, psS, psB, psA, sm_p, blk_p, work_p, s2T_p, s2c_p,
                      s2_p, s1T_p, s1stage_p, consts):
            _pool.release()

    return nc


def prepare_in_maps(inputs):
    bf = ml_dtypes.bfloat16
    f32 = np.float32

    def a(x, dt=f32):
        return np.ascontiguousarray(np.asarray(x), dtype=dt)

    W0 = a(inputs["W0"]); Wg = a(inputs["Wg"])
    Ws1 = a(inputs["Ws1"]); Ws2 = a(inputs["Ws2"])
    Wq = a(inputs["Wq"]); Wk = a(inputs["Wk"]); Wv = a(inputs["Wv"])
    Wc1 = a(inputs["Wc1"]); Wc2 = a(inputs["Wc2"]); Wc3 = a(inputs["Wc3"])

    wqk = np.ascontiguousarray((Wq @ Wk.T), dtype=f32)          # [64, 64]
    wvc = np.ascontiguousarray(Wv @ Wc1[128:192, :], dtype=f32)  # [64, 128]
    # block-diagonal Ws1: ws1_blk[j', q, 64*r + d] = Ws1[j, d] for rows
    # j' = 7*(2q+r)+j  (n' = 2q+r local neighbour within a 4-group)
    ws1_blk = np.zeros((28, 2, 128), dtype=f32)
    for q in range(2):
        for r in range(2):
            for j in range(7):
                ws1_blk[7 * (2 * q + r) + j, q, 64 * r : 64 * r + 64] = Ws1[j]
    ws2_blk = np.zeros((128, 128), dtype=f32)
    ws2_blk[0:64, 0:64] = Ws2
    ws2_blk[64:128, 64:128] = Ws2

    def col(x, n=None):
        v = np.asarray(x, dtype=f32).reshape(-1, 1)
        return np.ascontiguousarray(v)

    b0bg = np.concatenate([col(inputs["b0"]), col(inputs["bg"])], axis=0)
    bs1_rep = np.concatenate([col(inputs["bs1"])] * 2, axis=0)
    bs2_rep = np.concatenate([col(inputs["bs2"])] * 2, axis=0)

    wc3_pad = np.asarray(Wc3, dtype=f32)                          # [128, 2]

    state0 = a(inputs["state0"]); state1 = a(inputs["state1"])
    state2 = a(inputs["state2"])

    shared = {
        "wg_bf": a(Wg, bf),
        "ws1_blk": a(ws1_blk, bf),
        "ws2_blk": a(ws2_blk, bf),
        "w0": W0,
        "wqk": wqk,
        "wc1a": np.ascontiguousarray(Wc1[0:128, :], dtype=f32),
        "wvc": wvc,
        "wc2": Wc2,
        "wc3": wc3_pad,
        "b0bg": b0bg,
        "bs1_rep": bs1_rep,
        "bs2_rep": bs2_rep,
        "bc1": col(inputs["bc1"]),
        "bc2": col(inputs["bc2"]),
        "bc3": col(inputs["bc3"]),
    }
    in_maps = []
    for i in range(N_CORES):
        m = dict(shared)
        sl = slice(i * RPC, (i + 1) * RPC)
        m["state0"] = state0[sl]
        m["state1"] = state1[sl]
        m["state2"] = state2[sl]
        in_maps.append(m)
    return in_maps


_NC_CACHE = {}


def get_nc():
    if "nc" not in _NC_CACHE:
        nc = build()
        nc.finalize()
        _NC_CACHE["nc"] = nc
    return _NC_CACHE["nc"]


def kernel(**inputs):
    nc = get_nc()
    in_maps = prepare_in_maps(inputs)
    trace = bool(int(os.environ.get("K_TRACE", "0")))
    try:
        res = run_bass_kernel_spmd(
            nc, in_maps, core_ids=list(range(N_CORES)), trace=trace
        )
    except ModuleNotFoundError:
        # NTFF profiling hook unavailable in this container; run untraced.
        res = run_bass_kernel_spmd(nc, in_maps, core_ids=list(range(N_CORES)))
    if res.exec_time_ns is not None:
        print(f"HW exec time: {res.exec_time_ns} ns")
    parts = [np.asarray(res.results[i]["out"], dtype=np.float32).T for i in range(N_CORES)]
    return np.ascontiguousarray(np.concatenate(parts, axis=0))



# revision 15
# speedup vs baseline: 1.0817x; 1.0817x over previous
"""Trainium2 Bass kernel for nn_ActorNetwork (gnn_message_passing).

Pure data-parallel across 8 NeuronCores: each core processes 8192 of the
65536 batch rows; small weights are replicated.

v2 layout: feature-major throughout, with the attention computed in a fused
(neighbor-pair, macro-column) layout that never transposes i2 back to
batch-major:
  - i1/i2 as 4 wide matmuls each (neighbor-pairs packed on partitions,
    (o,b) = 512 macro columns streamed).
  - score_n[b] = sum_d i2 * q2 via one elementwise multiply (q2 produced
    partition-duplicated for free by a duplicated-column Wq*Wk^T stationary)
    and per-pair ones-matmul partition reductions, accumulated in PSUM on
    top of the PE-transposed -1e30 mask rows -> masked scores directly.
  - softmax over an [8, 512] tile (exp on Act, denominator via ones-matmul,
    reciprocal-dup via 1x8 matmul, one multiply -> alpha).
  - alpha broadcast back to the (r,d) partition layout via 4 selector
    matmuls; weighted i2 (cmul) feeds h1 directly through a row-duplicated
    Wvc stationary -- the attention output is never materialized.
Elementwise/copy work is spread across DVE / Act / Pool; all wide moving
operands are bf16 (DVE 2x mode, PE 1 cyc/col).
"""

import os

import numpy as np
import ml_dtypes

import concourse.bass as bass
import concourse.tile as tile
from concourse import bacc
from concourse import mybir
from concourse.bass_utils import run_bass_kernel_spmd
from concourse.masks import make_identity

F32 = mybir.dt.float32
F32R = mybir.dt.float32r
BF16 = mybir.dt.bfloat16

N_CORES = 8
B_FULL = 65536
RPC = B_FULL // N_CORES        # rows per core = 8192
MACRO = 512                    # batch rows per macro tile
P = 128

Relu = mybir.ActivationFunctionType.Relu
Tanh = mybir.ActivationFunctionType.Tanh
Exp = mybir.ActivationFunctionType.Exp
Alu = mybir.AluOpType
AX = mybir.AxisListType


def build(rpc=RPC, macro=MACRO):
    nm = rpc // macro          # macro tiles per core
    nb = macro // P            # 128-row blocks per macro tile

    nc = bacc.Bacc()

    s0 = nc.declare_dram_parameter("state0", [rpc, 6], F32R, isOutput=False)
    s1 = nc.declare_dram_parameter("state1", [rpc, 1024], F32, isOutput=False)
    s2 = nc.declare_dram_parameter("state2", [rpc, 8, 7], F32, isOutput=False)
    wg_d = nc.declare_dram_parameter("wg_bf", [1024, 64], BF16, isOutput=False)
    ws1_d = nc.declare_dram_parameter("ws1_blk", [56, 4, 128], BF16, isOutput=False)
    ws2_d = nc.declare_dram_parameter("ws2_blk", [128, 128], BF16, isOutput=False)
    w0_d = nc.declare_dram_parameter("w0", [6, 64], F32R, isOutput=False)
    wqk_d = nc.declare_dram_parameter("wqk_dup", [64, 128], F32R, isOutput=False)
    wc1a_d = nc.declare_dram_parameter("wc1a", [128, 128], F32R, isOutput=False)
    wvc_d = nc.declare_dram_parameter("wvc_dup", [128, 128], F32R, isOutput=False)
    wc2_d = nc.declare_dram_parameter("wc2", [128, 128], F32R, isOutput=False)
    wc3_d = nc.declare_dram_parameter("wc3", [128, 2], F32R, isOutput=False)
    sel_d = nc.declare_dram_parameter("sel", [8, 4, 128], F32R, isOutput=False)
    ones2_d = nc.declare_dram_parameter("scoresel", [128, 4, 8], F32R, isOutput=False)
    ones8_d = nc.declare_dram_parameter("ones8", [8, 1], F32R, isOutput=False)
    ones18_d = nc.declare_dram_parameter("ones18", [1, 8], F32R, isOutput=False)
    b0bg_d = nc.declare_dram_parameter("b0bg", [128, 1], F32, isOutput=False)
    bs1_d = nc.declare_dram_parameter("bs1_rep", [128, 1], F32, isOutput=False)
    bs2_d = nc.declare_dram_parameter("bs2_rep", [128, 1], F32, isOutput=False)
    bc1_d = nc.declare_dram_parameter("bc1", [128, 1], F32, isOutput=False)
    bc2_d = nc.declare_dram_parameter("bc2", [128, 1], F32, isOutput=False)
    bc3_d = nc.declare_dram_parameter("bc3", [2, 1], F32, isOutput=False)
    out_d = nc.declare_dram_parameter("out", [2, rpc], F32, isOutput=True)

    with tile.TileContext(nc) as tc:
        consts = tc.alloc_tile_pool(name="consts", bufs=1)
        s1stage_p = tc.alloc_tile_pool(name="s1stage", bufs=2)
        s1T_p = tc.alloc_tile_pool(name="s1T", bufs=2)
        s2_p = tc.alloc_tile_pool(name="s2", bufs=2)
        work_p = tc.alloc_tile_pool(name="work", bufs=2)
        blk_p = tc.alloc_tile_pool(name="blk", bufs=2)
        sm_p = tc.alloc_tile_pool(name="sm", bufs=2)
        psS = tc.alloc_tile_pool(name="psS", bufs=2, space="PSUM")
        psA = tc.alloc_tile_pool(name="psA", bufs=2, space="PSUM")
        psB = tc.alloc_tile_pool(name="psB", bufs=2, space="PSUM")
        psC = tc.alloc_tile_pool(name="psC", bufs=2, space="PSUM")

        # ---- constants / weights to SBUF ----
        wg_sb = consts.tile([P, 8, 64], BF16)
        nc.sync.dma_start(wg_sb, wg_d.rearrange("(c p) m -> p c m", p=P))
        ws1_sb = consts.tile([56, 4, P], BF16)
        nc.sync.dma_start(ws1_sb, ws1_d[:, :, :])
        ws2_sb = consts.tile([P, P], BF16)
        nc.sync.dma_start(ws2_sb, ws2_d[:, :])
        w0_sb = consts.tile([6, 64], F32R)
        nc.sync.dma_start(w0_sb, w0_d[:, :])
        wqk_sb = consts.tile([64, 128], F32R)
        nc.sync.dma_start(wqk_sb, wqk_d[:, :])
        wc1a_sb = consts.tile([P, 128], F32R)
        nc.sync.dma_start(wc1a_sb, wc1a_d[:, :])
        wvc_sb = consts.tile([P, 128], F32R)
        nc.sync.dma_start(wvc_sb, wvc_d[:, :])
        wc2_sb = consts.tile([P, 128], F32R)
        nc.sync.dma_start(wc2_sb, wc2_d[:, :])
        wc3_sb = consts.tile([P, 2], F32R)
        nc.sync.dma_start(wc3_sb, wc3_d[:, :])
        sel_sb = consts.tile([8, 4, P], F32R)
        nc.sync.dma_start(sel_sb, sel_d[:, :, :])
        ones2_sb = consts.tile([P, 4, 8], F32R)
        nc.sync.dma_start(ones2_sb, ones2_d[:, :, :])
        ones8_sb = consts.tile([8, 1], F32R)
        nc.sync.dma_start(ones8_sb, ones8_d[:, :])
        ones18_sb = consts.tile([1, 8], F32R)
        nc.sync.dma_start(ones18_sb, ones18_d[:, :])

        b0bg_sb = consts.tile([P, 1], F32)
        nc.sync.dma_start(b0bg_sb, b0bg_d[:, :])
        bs1_sb = consts.tile([P, 1], F32)
        nc.sync.dma_start(bs1_sb, bs1_d[:, :])
        bs2_sb = consts.tile([P, 1], F32)
        nc.sync.dma_start(bs2_sb, bs2_d[:, :])
        bc1_sb = consts.tile([P, 1], F32)
        nc.sync.dma_start(bc1_sb, bc1_d[:, :])
        bc2_sb = consts.tile([P, 1], F32)
        nc.sync.dma_start(bc2_sb, bc2_d[:, :])
        bc3_sb = consts.tile([2, 1], F32)
        nc.sync.dma_start(bc3_sb, bc3_d[:, :])

        # state0^T loaded once for the whole core (tiny, strided DMA)
        s0T_sb = consts.tile([6, rpc], F32R)
        with nc.allow_non_contiguous_dma(reason="tiny state0 transpose load"):
            nc.sync.dma_start(s0T_sb, s0.rearrange("b f -> f b"))

        ident_f = consts.tile([P, P], F32)
        make_identity(nc, ident_f)
        ident_b = consts.tile([P, P], BF16)
        nc.vector.tensor_copy(ident_b, ident_f)

        out_sb = consts.tile([2, rpc], F32)

        for ms in range(nm):
            row0 = ms * macro
            # ---- streamed loads ----
            s1_stage = s1stage_p.tile([P, nb, 1024], BF16, tag="s1stage")
            nc.gpsimd.dma_start(
                s1_stage,
                s1[row0 : row0 + macro, :].rearrange("(o p) f -> p o f", p=P),
            )
            s2_t = s2_p.tile([P, nb, 8, 7], F32, tag="s2")
            nc.sync.dma_start(
                s2_t,
                s2[row0 : row0 + macro, :, :].rearrange("(o p) n j -> p o n j", p=P),
            )

            # ---- state1 transpose on PE, moves spread over engines ----
            s1T = s1T_p.tile([P, 8, nb, P], BF16, tag="s1T")
            for o in range(nb):
                t_ps = psS.tile([P, 8, P], BF16, tag="psS")
                for c in range(8):
                    nc.tensor.transpose(
                        t_ps[:, c, :],
                        s1_stage[:, o, c * P : (c + 1) * P],
                        ident_b,
                    )
                dst = s1T[:, :, o, :]
                if o % 2 == 0:
                    nc.vector.tensor_copy(dst, t_ps)
                else:
                    nc.scalar.copy(dst, t_ps)

            # ---- own_e / env_e packed in one [128, 512] psum ----
            eo_ps = psB.tile([P, macro], F32, tag="psB")
            nc.tensor.matmul(
                eo_ps[0:64, :], w0_sb, s0T_sb[:, row0 : row0 + macro],
                start=True, stop=True,
            )
            for c in range(8):
                nc.tensor.matmul(
                    eo_ps[64:128, :], wg_sb[:, c], s1T[:, c],
                    start=(c == 0), stop=(c == 7), tile_position=(0, 64),
                )
            concatA = work_p.tile([P, macro], F32R, tag="concatA")
            nc.scalar.activation(concatA, eo_ps, Relu, bias=b0bg_sb)

            # ---- q2 = (WqWk^T)^T own, partition-duplicated via dup'd cols ----
            q2_ps = psB.tile([P, macro], F32, tag="psB")
            nc.tensor.matmul(q2_ps, wqk_sb, concatA[0:64, :], start=True, stop=True)
            q2_sb = work_p.tile([P, macro], F32R, tag="q2")
            nc.scalar.copy(q2_sb, q2_ps)

            # ---- state2 pack (bf16) + transpose to [56, (o,b)] ----
            s2c = sm_p.tile([P, nb, 56], BF16, tag="s2c")
            nc.gpsimd.tensor_copy(s2c, s2_t.rearrange("p o n j -> p o (n j)"))
            s2T_ps = psS.tile([P, nb, P], BF16, tag="psS")
            for o in range(nb):
                nc.tensor.transpose(s2T_ps[0:56, o, :], s2c[:, o, :], ident_b)
            s2T_sb = sm_p.tile([56, nb, P], BF16, tag="s2T")
            nc.vector.tensor_copy(s2T_sb, s2T_ps[0:56])

            # ---- mask rows: -1e30 where sum_j s2 == 0 ----
            msum = sm_p.tile([P, nb, 8], F32, tag="msum")
            nc.vector.reduce_sum(msum, s2_t, axis=AX.X)
            negm = sm_p.tile([P, nb, 8], F32, tag="negm")
            nc.vector.tensor_scalar(negm, msum, 0.0, -1e30, Alu.is_equal, Alu.mult)

            # ---- intru1/intru2: 4 wide matmuls each ----
            i1_sb = blk_p.tile([P, 4, macro], BF16, tag="i1")
            for m in range(4):
                i1_ps = psA.tile([P, macro], F32, tag="psA")
                nc.tensor.matmul(i1_ps, ws1_sb[:, m], s2T_sb, start=True, stop=True)
                dst = i1_sb[:, m, :]
                if m == 0:
                    nc.vector.tensor_scalar(dst, i1_ps, bs1_sb, 0.0, Alu.add, Alu.max)
                elif m == 1:
                    nc.scalar.activation(dst, i1_ps, Relu, bias=bs1_sb)
                elif m == 2:
                    nc.scalar.activation(dst, i1_ps, Relu, bias=bs1_sb)
                else:
                    nc.vector.tensor_scalar(dst, i1_ps, bs1_sb, 0.0, Alu.add, Alu.max)

            i2_sb = blk_p.tile([P, 4, macro], BF16, tag="i2")
            for m in range(4):
                i2_ps = psA.tile([P, macro], F32, tag="psA")
                nc.tensor.matmul(i2_ps, ws2_sb, i1_sb[:, m, :], start=True, stop=True)
                dst = i2_sb[:, m, :]
                if m == 0:
                    nc.scalar.activation(dst, i2_ps, Relu, bias=bs2_sb)
                elif m == 1:
                    nc.vector.tensor_scalar(dst, i2_ps, bs2_sb, 0.0, Alu.add, Alu.max)
                elif m == 2:
                    nc.vector.tensor_scalar(dst, i2_ps, bs2_sb, 0.0, Alu.add, Alu.max)
                else:
                    nc.scalar.activation(dst, i2_ps, Relu, bias=bs2_sb)

            # ---- qk = i2 * q2 (bf16, one wide DVE op) ----
            qk_sb = blk_p.tile([P, 4, macro], F32R, tag="qk")
            nc.gpsimd.tensor_tensor(
                qk_sb, i2_sb,
                q2_sb[:, None, :].to_broadcast((P, 4, macro)),
                Alu.mult,
            )

            # ---- masked scores in PSUM: transpose mask in, accumulate dots ----
            nm_ps = psC.tile([8, macro], F32, tag="psC")
            for o in range(nb):
                nc.tensor.transpose(
                    nm_ps[:, o * P : (o + 1) * P], negm[:, o, :], ident_f
                )
            nm_sb = sm_p.tile([8, macro], BF16, tag="nm")
            nc.scalar.copy(nm_sb, nm_ps)
            sc_ps = psC.tile([8, macro], F32, tag="psC")
            for m in range(4):
                nc.tensor.matmul(
                    sc_ps, ones2_sb[:, m, :], qk_sb[:, m, :],
                    start=(m == 0), stop=(m == 3),
                )
            nc.vector.tensor_tensor(sc_ps, sc_ps, nm_sb, Alu.add)

            # ---- softmax over the 8 slots ----
            p8_sb = sm_p.tile([8, macro], F32R, tag="p8")
            nc.scalar.activation(p8_sb, sc_ps, Exp, scale=0.125)
            den_ps = psC.tile([1, macro], F32, tag="psC")
            nc.tensor.matmul(den_ps, ones8_sb, p8_sb, start=True, stop=True)
            rs_sb = sm_p.tile([1, macro], F32R, tag="rs")
            with nc.allow_low_precision(reason="f32r reciprocal, 19-bit ok"):
                nc.vector.reciprocal(rs_sb, den_ps)
            dup_ps = psC.tile([8, macro], F32, tag="psC")
            nc.tensor.matmul(dup_ps, ones18_sb, rs_sb, start=True, stop=True)
            alpha_sb = sm_p.tile([8, macro], F32R, tag="alpha")
            nc.vector.tensor_tensor(alpha_sb, p8_sb, dup_ps, Alu.mult)

            # ---- weighted i2 (cmul); attention output folded into h1 ----
            cmul_sb = blk_p.tile([P, 4, macro], F32R, tag="cmul")
            for m in range(4):
                abc_ps = psA.tile([P, macro], F32, tag="psA")
                nc.tensor.matmul(abc_ps, sel_sb[:, m], alpha_sb, start=True, stop=True)
                dst = cmul_sb[:, m, :]
                nc.vector.tensor_tensor(dst, i2_sb[:, m, :], abc_ps, Alu.mult)

            # ---- head: h1 = relu(Wc1a^T concat + sum_m Wvc_dup^T cmul_m) ----
            h1_ps = psB.tile([P, macro], F32, tag="psB")
            nc.tensor.matmul(h1_ps, wc1a_sb, concatA, start=True, stop=False)
            for m in range(4):
                nc.tensor.matmul(
                    h1_ps, wvc_sb, cmul_sb[:, m, :],
                    start=False, stop=(m == 3),
                )
            h1_sb = work_p.tile([P, macro], F32R, tag="h1")
            nc.vector.tensor_scalar(h1_sb, h1_ps, bc1_sb, 0.0, Alu.add, Alu.max)

            h2_ps = psB.tile([P, macro], F32, tag="psB")
            nc.tensor.matmul(h2_ps, wc2_sb, h1_sb, start=True, stop=True)
            h2_sb = work_p.tile([P, macro], F32R, tag="h2")
            nc.scalar.activation(h2_sb, h2_ps, Relu, bias=bc2_sb)

            o_ps = psB.tile([2, macro], F32, tag="psB")
            nc.tensor.matmul(o_ps, wc3_sb, h2_sb, start=True, stop=True)
            nc.scalar.activation(
                out_sb[:, row0 : row0 + macro], o_ps, Tanh, bias=bc3_sb
            )

        nc.sync.dma_start(out_d[:, :], out_sb)

        for _pool in (psC, psB, psA, psS, sm_p, blk_p, work_p, s2_p, s1T_p,
                      s1stage_p, consts):
            _pool.release()

    return nc


def prepare_in_maps(inputs):
    bf = ml_dtypes.bfloat16
    f32 = np.float32

    def a(x, dt=f32):
        return np.ascontiguousarray(np.asarray(x), dtype=dt)

    W0 = a(inputs["W0"]); Wg = a(inputs["Wg"])
    Ws1 = a(inputs["Ws1"]); Ws2 = a(inputs["Ws2"])
    Wq = a(inputs["Wq"]); Wk = a(inputs["Wk"]); Wv = a(inputs["Wv"])
    Wc1 = a(inputs["Wc1"]); Wc2 = a(inputs["Wc2"]); Wc3 = a(inputs["Wc3"])

    wqk = Wq @ Wk.T                                   # [64, 64]
    wqk_dup = np.concatenate([wqk, wqk], axis=1)      # [64, 128]
    wvc = Wv @ Wc1[128:192, :]                        # [64, 128]
    wvc_dup = np.concatenate([wvc, wvc], axis=0)      # [128, 128]

    # ws1_blk[7n+j, m, 64r+d] = Ws1[j, d] if n == 2m+r
    ws1_blk = np.zeros((56, 4, 128), dtype=f32)
    sel = np.zeros((8, 4, 128), dtype=f32)
    for n in range(8):
        m, r = n // 2, n % 2
        ws1_blk[7 * n : 7 * n + 7, m, 64 * r : 64 * r + 64] = Ws1
        sel[n, m, 64 * r : 64 * r + 64] = 1.0
    ws2_blk = np.zeros((128, 128), dtype=f32)
    ws2_blk[0:64, 0:64] = Ws2
    ws2_blk[64:128, 64:128] = Ws2

    # scoresel[64r+d, m, n] = 1 iff n == 2m+r (score partition reduce)
    scoresel = np.zeros((128, 4, 8), dtype=f32)
    for n in range(8):
        m, r = n // 2, n % 2
        scoresel[64 * r : 64 * r + 64, m, n] = 1.0
    ones8 = np.ones((8, 1), dtype=f32)
    ones18 = np.ones((1, 8), dtype=f32)

    def col(x):
        return np.ascontiguousarray(np.asarray(x, dtype=f32).reshape(-1, 1))

    b0bg = np.concatenate([col(inputs["b0"]), col(inputs["bg"])], axis=0)
    bs1_rep = np.concatenate([col(inputs["bs1"])] * 2, axis=0)
    bs2_rep = np.concatenate([col(inputs["bs2"])] * 2, axis=0)

    state0 = a(inputs["state0"]); state1 = a(inputs["state1"])
    state2 = a(inputs["state2"])

    shared = {
        "wg_bf": a(Wg, bf),
        "ws1_blk": a(ws1_blk, bf),
        "ws2_blk": a(ws2_blk, bf),
        "w0": W0,
        "wqk_dup": wqk_dup.astype(f32),
        "wc1a": a(Wc1[0:128, :]),
        "wvc_dup": a(wvc_dup),
        "wc2": a(Wc2),
        "wc3": a(Wc3),
        "sel": sel,
        "scoresel": scoresel,
        "ones8": ones8,
        "ones18": ones18,
        "b0bg": b0bg,
        "bs1_rep": bs1_rep,
        "bs2_rep": bs2_rep,
        "bc1": col(inputs["bc1"]),
        "bc2": col(inputs["bc2"]),
        "bc3": col(inputs["bc3"]),
    }
    in_maps = []
    for i in range(N_CORES):
        m = dict(shared)
        sl = slice(i * RPC, (i + 1) * RPC)
        m["state0"] = state0[sl]
        m["state1"] = state1[sl]
        m["state2"] = state2[sl]
        in_maps.append(m)
    return in_maps


_NC_CACHE = {}


def get_nc():
    if "nc" not in _NC_CACHE:
        nc = build()
        nc.finalize()
        _NC_CACHE["nc"] = nc
    return _NC_CACHE["nc"]


def kernel(**inputs):
    nc = get_nc()
    in_maps = prepare_in_maps(inputs)
    trace = bool(int(os.environ.get("K_TRACE", "0")))
    try:
        res = run_bass_kernel_spmd(
            nc, in_maps, core_ids=list(range(N_CORES)), trace=trace
        )
    except ModuleNotFoundError:
        res = run_bass_kernel_spmd(nc, in_maps, core_ids=list(range(N_CORES)))
    if res.exec_time_ns is not None:
        print(f"HW exec time: {res.exec_time_ns} ns")
    parts = [np.asarray(res.results[i]["out"], dtype=np.float32).T for i in range(N_CORES)]
    return np.ascontiguousarray(np.concatenate(parts, axis=0))


# revision 20
# speedup vs baseline: 1.3585x; 1.2558x over previous
"""Trainium2 Bass kernel for nn_ActorNetwork (gnn_message_passing).

Pure data-parallel across 8 NeuronCores: each core processes 8192 of the
65536 batch rows; small weights are replicated.

v2 layout: feature-major throughout, with the attention computed in a fused
(neighbor-pair, macro-column) layout that never transposes i2 back to
batch-major:
  - i1/i2 as 4 wide matmuls each (neighbor-pairs packed on partitions,
    (o,b) = 512 macro columns streamed).
  - score_n[b] = sum_d i2 * q2 via one elementwise multiply (q2 produced
    partition-duplicated for free by a duplicated-column Wq*Wk^T stationary)
    and per-pair ones-matmul partition reductions, accumulated in PSUM on
    top of the PE-transposed -1e30 mask rows -> masked scores directly.
  - softmax over an [8, 512] tile (exp on Act, denominator via ones-matmul,
    reciprocal-dup via 1x8 matmul, one multiply -> alpha).
  - alpha broadcast back to the (r,d) partition layout via 4 selector
    matmuls; weighted i2 (cmul) feeds h1 directly through a row-duplicated
    Wvc stationary -- the attention output is never materialized.
Elementwise/copy work is spread across DVE / Act / Pool; all wide moving
operands are bf16 (DVE 2x mode, PE 1 cyc/col).
"""

import os

import numpy as np
import ml_dtypes

import concourse.bass as bass
import concourse.tile as tile
from concourse import bacc
from concourse import mybir
from concourse.bass_utils import run_bass_kernel_spmd
from concourse.masks import make_identity

F32 = mybir.dt.float32
F32R = mybir.dt.float32r
BF16 = mybir.dt.bfloat16

N_CORES = 8
B_FULL = 65536
RPC = B_FULL // N_CORES        # rows per core = 8192
MACRO = 512                    # batch rows per macro tile
P = 128

Relu = mybir.ActivationFunctionType.Relu
Tanh = mybir.ActivationFunctionType.Tanh
Exp = mybir.ActivationFunctionType.Exp
Alu = mybir.AluOpType
AX = mybir.AxisListType


def build(rpc=RPC, macro=MACRO):
    nm = rpc // macro          # macro tiles per core
    nb = macro // P            # 128-row blocks per macro tile

    nc = bacc.Bacc()

    s0 = nc.declare_dram_parameter("state0", [rpc, 6], F32R, isOutput=False)
    s1 = nc.declare_dram_parameter("state1", [rpc, 1024], F32, isOutput=False)
    s2 = nc.declare_dram_parameter("state2", [rpc, 8, 7], F32, isOutput=False)
    wg_d = nc.declare_dram_parameter("wg_bf", [1024, 64], BF16, isOutput=False)
    ws1_d = nc.declare_dram_parameter("ws1_blk", [56, 4, 128], BF16, isOutput=False)
    ws2_d = nc.declare_dram_parameter("ws2_blk", [128, 128], BF16, isOutput=False)
    w0_d = nc.declare_dram_parameter("w0", [6, 64], F32R, isOutput=False)
    wqk_d = nc.declare_dram_parameter("wqk_dup", [64, 128], BF16, isOutput=False)
    wc1a_d = nc.declare_dram_parameter("wc1a", [128, 128], BF16, isOutput=False)
    wvc_d = nc.declare_dram_parameter("wvc_dup", [128, 128], BF16, isOutput=False)
    wc2_d = nc.declare_dram_parameter("wc2", [128, 128], BF16, isOutput=False)
    wc3_d = nc.declare_dram_parameter("wc3", [128, 2], BF16, isOutput=False)
    sel_d = nc.declare_dram_parameter("sel", [8, 4, 128], BF16, isOutput=False)
    ones2_d = nc.declare_dram_parameter("scoresel", [128, 4, 8], BF16, isOutput=False)
    ones8_d = nc.declare_dram_parameter("ones8", [8, 1], BF16, isOutput=False)
    ones18_d = nc.declare_dram_parameter("ones18", [1, 8], F32R, isOutput=False)
    jsum_d = nc.declare_dram_parameter("jsum", [56, 8], BF16, isOutput=False)
    b0bg_d = nc.declare_dram_parameter("b0bg", [128, 1], F32, isOutput=False)
    bs1_d = nc.declare_dram_parameter("bs1_rep", [128, 1], F32, isOutput=False)
    bs2_d = nc.declare_dram_parameter("bs2_rep", [128, 1], F32, isOutput=False)
    bc1_d = nc.declare_dram_parameter("bc1", [128, 1], F32, isOutput=False)
    bc2_d = nc.declare_dram_parameter("bc2", [128, 1], F32, isOutput=False)
    bc3_d = nc.declare_dram_parameter("bc3", [2, 1], F32, isOutput=False)
    out_d = nc.declare_dram_parameter("out", [2, rpc], F32, isOutput=True)

    with tile.TileContext(nc) as tc:
        consts = tc.alloc_tile_pool(name="consts", bufs=1)
        s1stage_p = tc.alloc_tile_pool(name="s1stage", bufs=2)
        s1T_p = tc.alloc_tile_pool(name="s1T", bufs=2)
        s2_p = tc.alloc_tile_pool(name="s2", bufs=2)
        work_p = tc.alloc_tile_pool(name="work", bufs=3)
        blk_p = tc.alloc_tile_pool(name="blk", bufs=3)
        sm_p = tc.alloc_tile_pool(name="sm", bufs=3)
        psS = tc.alloc_tile_pool(name="psS", bufs=2, space="PSUM")
        psA = tc.alloc_tile_pool(name="psA", bufs=2, space="PSUM")
        psB = tc.alloc_tile_pool(name="psB", bufs=1, space="PSUM")
        psB2 = tc.alloc_tile_pool(name="psB2", bufs=1, space="PSUM")
        psC = tc.alloc_tile_pool(name="psC", bufs=2, space="PSUM")

        # ---- constants / weights to SBUF ----
        wg_sb = consts.tile([P, 8, 64], BF16)
        nc.sync.dma_start(wg_sb, wg_d.rearrange("(c p) m -> p c m", p=P))
        ws1_sb = consts.tile([56, 4, P], BF16)
        nc.sync.dma_start(ws1_sb, ws1_d[:, :, :])
        ws2_sb = consts.tile([P, P], BF16)
        nc.sync.dma_start(ws2_sb, ws2_d[:, :])
        w0_sb = consts.tile([6, 64], F32R)
        nc.sync.dma_start(w0_sb, w0_d[:, :])
        wqk_sb = consts.tile([64, 128], BF16)
        nc.sync.dma_start(wqk_sb, wqk_d[:, :])
        wc1a_sb = consts.tile([P, 128], BF16)
        nc.sync.dma_start(wc1a_sb, wc1a_d[:, :])
        wvc_sb = consts.tile([P, 128], BF16)
        nc.sync.dma_start(wvc_sb, wvc_d[:, :])
        wc2_sb = consts.tile([P, 128], BF16)
        nc.sync.dma_start(wc2_sb, wc2_d[:, :])
        wc3_sb = consts.tile([P, 2], BF16)
        nc.sync.dma_start(wc3_sb, wc3_d[:, :])
        sel_sb = consts.tile([8, 4, P], BF16)
        nc.sync.dma_start(sel_sb, sel_d[:, :, :])
        ones2_sb = consts.tile([P, 4, 8], BF16)
        nc.sync.dma_start(ones2_sb, ones2_d[:, :, :])
        ones8_sb = consts.tile([8, 1], BF16)
        nc.sync.dma_start(ones8_sb, ones8_d[:, :])
        ones18_sb = consts.tile([1, 8], F32R)
        nc.sync.dma_start(ones18_sb, ones18_d[:, :])
        jsum_sb = consts.tile([56, 8], BF16)
        nc.sync.dma_start(jsum_sb, jsum_d[:, :])

        b0bg_sb = consts.tile([P, 1], F32)
        nc.sync.dma_start(b0bg_sb, b0bg_d[:, :])
        bs1_sb = consts.tile([P, 1], F32)
        nc.sync.dma_start(bs1_sb, bs1_d[:, :])
        bs2_sb = consts.tile([P, 1], F32)
        nc.sync.dma_start(bs2_sb, bs2_d[:, :])
        bc1_sb = consts.tile([P, 1], F32)
        nc.sync.dma_start(bc1_sb, bc1_d[:, :])
        bc2_sb = consts.tile([P, 1], F32)
        nc.sync.dma_start(bc2_sb, bc2_d[:, :])
        bc3_sb = consts.tile([2, 1], F32)
        nc.sync.dma_start(bc3_sb, bc3_d[:, :])

        # state0^T loaded once for the whole core (tiny, strided DMA)
        s0T_sb = consts.tile([6, rpc], F32R)
        with nc.allow_non_contiguous_dma(reason="tiny state0 transpose load"):
            nc.sync.dma_start(s0T_sb, s0.rearrange("b f -> f b"))

        ident_f = consts.tile([P, P], F32)
        make_identity(nc, ident_f)
        ident_b = consts.tile([P, P], BF16)
        nc.vector.tensor_copy(ident_b, ident_f)

        out_sb = consts.tile([2, rpc], F32)

        def emit_front(ms):
            """Loads, state1 transposes, env/own, q2, state2 path, i1, i2, qk,
            mask rows. Returns state consumed by emit_back."""
            row0 = ms * macro
            s1_stage = s1stage_p.tile([P, nb, 1024], BF16, tag="s1stage")
            nc.gpsimd.dma_start(
                s1_stage,
                s1[row0 : row0 + macro, :].rearrange("(o p) f -> p o f", p=P),
            )
            s2_t = s2_p.tile([P, nb, 8, 7], F32, tag="s2")
            nc.sync.dma_start(
                s2_t,
                s2[row0 : row0 + macro, :, :].rearrange("(o p) n j -> p o n j", p=P),
            )

            # state2 pack (bf16) + transpose to [56, (o,b)] + mask rows
            s2c = sm_p.tile([P, nb, 56], BF16, tag="s2c")
            nc.gpsimd.tensor_copy(s2c, s2_t.rearrange("p o n j -> p o (n j)"))
            s2T_ps = psS.tile([P, nb, P], BF16, tag="psS")
            for o in range(nb):
                nc.tensor.transpose(s2T_ps[0:56, o, :], s2c[:, o, :], ident_b)
            s2T_sb = sm_p.tile([56, nb, P], BF16, tag="s2T")
            nc.vector.tensor_copy(s2T_sb, s2T_ps[0:56])

            nmsum_ps = psC.tile([8, macro], F32, tag="psC")
            nc.tensor.matmul(nmsum_ps, jsum_sb, s2T_sb, start=True, stop=True)
            nm_sb = sm_p.tile([8, macro], BF16, tag="nm")
            nc.vector.tensor_scalar(
                nm_sb, nmsum_ps, 0.0, -1e30, Alu.is_equal, Alu.mult
            )

            # state1 transpose on PE, moves spread over engines
            s1T = s1T_p.tile([P, 8, nb, P], BF16, tag="s1T")
            for o in range(nb):
                t_ps = psS.tile([P, 8, P], BF16, tag="psS")
                for c in range(8):
                    nc.tensor.transpose(
                        t_ps[:, c, :],
                        s1_stage[:, o, c * P : (c + 1) * P],
                        ident_b,
                    )
                dst = s1T[:, :, o, :]
                if o % 2 == 0:
                    nc.vector.tensor_copy(dst, t_ps)
                else:
                    nc.scalar.copy(dst, t_ps)

            # own_e / env_e packed in one [128, 512] psum
            eo_ps = psB.tile([P, macro], F32, tag="psB")
            nc.tensor.matmul(
                eo_ps[0:64, :], w0_sb, s0T_sb[:, row0 : row0 + macro],
                start=True, stop=True,
            )
            for c in range(8):
                nc.tensor.matmul(
                    eo_ps[64:128, :], wg_sb[:, c], s1T[:, c],
                    start=(c == 0), stop=(c == 7), tile_position=(0, 64),
                )
            concatA = work_p.tile([P, macro], BF16, tag="concatA")
            nc.scalar.activation(concatA, eo_ps, Relu, bias=b0bg_sb)

            # q2 = (WqWk^T)^T own, partition-duplicated via dup'd cols
            q2_ps = psB.tile([P, macro], F32, tag="psB")
            nc.tensor.matmul(q2_ps, wqk_sb, concatA[0:64, :], start=True, stop=True)
            q2_sb = work_p.tile([P, macro], BF16, tag="q2")
            nc.scalar.copy(q2_sb, q2_ps)

            # intru1/intru2: 4 wide matmuls each
            i1_sb = blk_p.tile([P, 4, macro], BF16, tag="i1")
            for m in range(4):
                i1_ps = psA.tile([P, macro], F32, tag="psA")
                nc.tensor.matmul(i1_ps, ws1_sb[:, m], s2T_sb, start=True, stop=True)
                dst = i1_sb[:, m, :]
                if m in (0, 3):
                    nc.vector.tensor_scalar(dst, i1_ps, bs1_sb, 0.0, Alu.add, Alu.max)
                else:
                    nc.scalar.activation(dst, i1_ps, Relu, bias=bs1_sb)

            i2_sb = blk_p.tile([P, 4, macro], BF16, tag="i2")
            for m in range(4):
                i2_ps = psA.tile([P, macro], F32, tag="psA")
                nc.tensor.matmul(i2_ps, ws2_sb, i1_sb[:, m, :], start=True, stop=True)
                dst = i2_sb[:, m, :]
                if m in (1, 2):
                    nc.vector.tensor_scalar(dst, i2_ps, bs2_sb, 0.0, Alu.add, Alu.max)
                else:
                    nc.scalar.activation(dst, i2_ps, Relu, bias=bs2_sb)

            # qk = i2 * q2 (3/4 on DVE, 1/4 on Pool)
            qk_sb = blk_p.tile([P, 4, macro], BF16, tag="qk")
            nc.vector.tensor_tensor(
                qk_sb[:, 0:3, :], i2_sb[:, 0:3, :],
                q2_sb[:, None, :].to_broadcast((P, 3, macro)),
                Alu.mult,
            )
            nc.gpsimd.tensor_tensor(
                qk_sb[:, 3, :], i2_sb[:, 3, :], q2_sb, Alu.mult
            )
            return dict(row0=row0, concatA=concatA, i2_sb=i2_sb, qk_sb=qk_sb,
                        nm_sb=nm_sb)

        def emit_back(st):
            """Scores, softmax, weighted i2, head, tanh."""
            row0 = st["row0"]
            concatA = st["concatA"]; i2_sb = st["i2_sb"]
            qk_sb = st["qk_sb"]; nm_sb = st["nm_sb"]

            sc_ps = psC.tile([8, macro], F32, tag="psC")
            for m in range(4):
                nc.tensor.matmul(
                    sc_ps, ones2_sb[:, m, :], qk_sb[:, m, :],
                    start=(m == 0), stop=(m == 3),
                )
            nc.vector.tensor_tensor(sc_ps, sc_ps, nm_sb, Alu.add)

            # softmax over the 8 slots
            p8_sb = sm_p.tile([8, macro], BF16, tag="p8")
            nc.scalar.activation(p8_sb, sc_ps, Exp, scale=0.125)
            den_ps = psC.tile([1, macro], F32, tag="psC")
            nc.tensor.matmul(den_ps, ones8_sb, p8_sb, start=True, stop=True)
            rs_sb = sm_p.tile([1, macro], F32R, tag="rs")
            with nc.allow_low_precision(reason="f32r reciprocal, 19-bit ok"):
                nc.vector.reciprocal(rs_sb, den_ps)
            dup_ps = psC.tile([8, macro], F32, tag="psC")
            nc.tensor.matmul(dup_ps, ones18_sb, rs_sb, start=True, stop=True)
            alpha_sb = sm_p.tile([8, macro], BF16, tag="alpha")
            nc.vector.tensor_tensor(alpha_sb, p8_sb, dup_ps, Alu.mult)

            # weighted i2 (cmul); attention output folded into h1
            cmul_sb = blk_p.tile([P, 4, macro], BF16, tag="cmul")
            for m in range(4):
                abc_ps = psA.tile([P, macro], F32, tag="psA")
                nc.tensor.matmul(abc_ps, sel_sb[:, m], alpha_sb, start=True, stop=True)
                nc.vector.tensor_tensor(
                    cmul_sb[:, m, :], i2_sb[:, m, :], abc_ps, Alu.mult
                )

            # head: h1 = relu(Wc1a^T concat + sum_m Wvc_dup^T cmul_m)
            h1_ps = psB2.tile([P, macro], F32, tag="psB2")
            nc.tensor.matmul(h1_ps, wc1a_sb, concatA, start=True, stop=False)
            for m in range(4):
                nc.tensor.matmul(
                    h1_ps, wvc_sb, cmul_sb[:, m, :],
                    start=False, stop=(m == 3),
                )
            h1_sb = work_p.tile([P, macro], BF16, tag="h1")
            nc.vector.tensor_scalar(h1_sb, h1_ps, bc1_sb, 0.0, Alu.add, Alu.max)

            h2_ps = psB2.tile([P, macro], F32, tag="psB2")
            nc.tensor.matmul(h2_ps, wc2_sb, h1_sb, start=True, stop=True)
            h2_sb = work_p.tile([P, macro], BF16, tag="h2")
            nc.scalar.activation(h2_sb, h2_ps, Relu, bias=bc2_sb)

            o_ps = psB2.tile([2, macro], F32, tag="psB2")
            nc.tensor.matmul(o_ps, wc3_sb, h2_sb, start=True, stop=True)
            nc.scalar.activation(
                out_sb[:, row0 : row0 + macro], o_ps, Tanh, bias=bc3_sb
            )

        # software pipeline: front(m+1) is emitted before back(m) so each
        # engine's in-order stream has independent work to fill dependency
        # stalls in the softmax/attention chain.
        pending = None
        for ms in range(nm):
            st = emit_front(ms)
            if pending is not None:
                emit_back(pending)
            pending = st
        emit_back(pending)

        nc.sync.dma_start(out_d[:, :], out_sb)

        for _pool in (psC, psB2, psB, psA, psS, sm_p, blk_p, work_p, s2_p, s1T_p,
                      s1stage_p, consts):
            _pool.release()

    return nc


def prepare_in_maps(inputs):
    bf = ml_dtypes.bfloat16
    f32 = np.float32

    def a(x, dt=f32):
        return np.ascontiguousarray(np.asarray(x), dtype=dt)

    W0 = a(inputs["W0"]); Wg = a(inputs["Wg"])
    Ws1 = a(inputs["Ws1"]); Ws2 = a(inputs["Ws2"])
    Wq = a(inputs["Wq"]); Wk = a(inputs["Wk"]); Wv = a(inputs["Wv"])
    Wc1 = a(inputs["Wc1"]); Wc2 = a(inputs["Wc2"]); Wc3 = a(inputs["Wc3"])

    wqk = Wq @ Wk.T                                   # [64, 64]
    wqk_dup = np.concatenate([wqk, wqk], axis=1)      # [64, 128]
    wvc = Wv @ Wc1[128:192, :]                        # [64, 128]
    wvc_dup = np.concatenate([wvc, wvc], axis=0)      # [128, 128]

    # ws1_blk[7n+j, m, 64r+d] = Ws1[j, d] if n == 2m+r
    ws1_blk = np.zeros((56, 4, 128), dtype=f32)
    sel = np.zeros((8, 4, 128), dtype=f32)
    for n in range(8):
        m, r = n // 2, n % 2
        ws1_blk[7 * n : 7 * n + 7, m, 64 * r : 64 * r + 64] = Ws1
        sel[n, m, 64 * r : 64 * r + 64] = 1.0
    ws2_blk = np.zeros((128, 128), dtype=f32)
    ws2_blk[0:64, 0:64] = Ws2
    ws2_blk[64:128, 64:128] = Ws2

    # scoresel[64r+d, m, n] = 1 iff n == 2m+r (score partition reduce)
    scoresel = np.zeros((128, 4, 8), dtype=f32)
    for n in range(8):
        m, r = n // 2, n % 2
        scoresel[64 * r : 64 * r + 64, m, n] = 1.0
    ones8 = np.ones((8, 1), dtype=f32)
    ones18 = np.ones((1, 8), dtype=f32)
    # jsum[7n+j, n'] = 1 iff n == n'  (per-neighbor feature sums for the mask)
    jsum = np.zeros((56, 8), dtype=f32)
    for n in range(8):
        jsum[7 * n : 7 * n + 7, n] = 1.0

    def col(x):
        return np.ascontiguousarray(np.asarray(x, dtype=f32).reshape(-1, 1))

    b0bg = np.concatenate([col(inputs["b0"]), col(inputs["bg"])], axis=0)
    bs1_rep = np.concatenate([col(inputs["bs1"])] * 2, axis=0)
    bs2_rep = np.concatenate([col(inputs["bs2"])] * 2, axis=0)

    state0 = a(inputs["state0"]); state1 = a(inputs["state1"])
    state2 = a(inputs["state2"])

    shared = {
        "wg_bf": a(Wg, bf),
        "ws1_blk": a(ws1_blk, bf),
        "ws2_blk": a(ws2_blk, bf),
        "w0": W0,
        "wqk_dup": a(wqk_dup, bf),
        "wc1a": a(Wc1[0:128, :], bf),
        "wvc_dup": a(wvc_dup, bf),
        "wc2": a(Wc2, bf),
        "wc3": a(Wc3, bf),
        "sel": a(sel, bf),
        "scoresel": a(scoresel, bf),
        "ones8": a(ones8, bf),
        "ones18": ones18,
        "jsum": a(jsum, bf),
        "b0bg": b0bg,
        "bs1_rep": bs1_rep,
        "bs2_rep": bs2_rep,
        "bc1": col(inputs["bc1"]),
        "bc2": col(inputs["bc2"]),
        "bc3": col(inputs["bc3"]),
    }
    in_maps = []
    for i in range(N_CORES):
        m = dict(shared)
        sl = slice(i * RPC, (i + 1) * RPC)
        m["state0"] = state0[sl]
        m["state1"] = state1[sl]
        m["state2"] = state2[sl]
        in_maps.append(m)
    return in_maps


_NC_CACHE = {}


def get_nc():
    if "nc" not in _NC_CACHE:
        nc = build()
        nc.finalize()
        _NC_CACHE["nc"] = nc
    return _NC_CACHE["nc"]


def kernel(**inputs):
    nc = get_nc()
    in_maps = prepare_in_maps(inputs)
    trace = bool(int(os.environ.get("K_TRACE", "0")))
    try:
        res = run_bass_kernel_spmd(
            nc, in_maps, core_ids=list(range(N_CORES)), trace=trace
        )
    except ModuleNotFoundError:
        res = run_bass_kernel_spmd(nc, in_maps, core_ids=list(range(N_CORES)))
    if res.exec_time_ns is not None:
        print(f"HW exec time: {res.exec_time_ns} ns")
    parts = [np.asarray(res.results[i]["out"], dtype=np.float32).T for i in range(N_CORES)]
    return np.ascontiguousarray(np.concatenate(parts, axis=0))
